# revision 13
# baseline (speedup 1.0000x reference)
"""Causal self-attention (B=4, T=2048, C=768, H=12, D=64) on 8 TRN2 NeuronCores.

Sharding: core = 2*b + hg. Data parallel over batch (4), tensor parallel over
heads (2 groups of 6). Each core computes qkv for its 6 heads, causal
attention, and a partial output projection (its heads' columns of w_proj);
the host sums the two partials per batch and adds b_proj.

Layout notes (per core):
  - xT   [768, 2048]  x[b] transposed on host (contraction dim on partitions)
  - kq   [128, 6, 2048] SBUF: f-tiles 0-2 = K^T feats, 3-5 = Q^T feats
  - v    [128, 16, 390] SBUF: token-major V, 65 cols/head (col 64 = ones so
         the attn@V matmul also produces the softmax denominator l)
  - scores computed transposed S^T[k, q] so no transposes are needed anywhere;
    softmax uses no max-subtraction (logits are O(10) for this problem) so
    P = exp(0.25 * QK^T_raw), Y^T_aug = V_aug^T @ P^T accumulated over k-tiles.
  - matmul operands are bf16 (fp32 PSUM accumulation); 1/l via a custom-DVE
    approx reciprocal (SBUF-in only!) + tiny f32r ones-outer-product broadcast.
  - S^T tiles are paired [128, 1024] so one ACT exp covers two k-tiles.

Scheduling (the performance core of this kernel):
  - The attention inner loop is ACT-paced: one exp pair is ~1.09us on the
    Activation engine vs 0.86us for its 4 matmuls (S-pair + Y-pair), so the
    PE needs ~1 independent filler matmul per half-pair to stay dense.
    Fillers (qkv for chunk qc+1, projection for finished chunks) are emitted
    one matmul at a time INSIDE each attention unit via a singleton generator
    stepper (never two fillers mid-flight -> bounded PSUM pool usage).
  - qkv chains carry deadlines (chunk tn's K/Q f-tile pair (fj,3+fj) is first
    read by unit (tn, 2*fj); V by unit (tn,0)) and are force-drained right
    before the unit that needs them, so a late filler can never end up behind
    the matmul that consumes it in the in-order PE stream.
  - startup: biases (bf16, replicated host-side) + causal mask come as plain
    DMA inputs; wkq is split per c-tile into an f-tile-0/3 part (needed by the
    first attention unit) and the rest, and the three DMA queues (~100GB/s
    each, FIFO) are ordered so the first kq chain is fed within ~2us of the
    preamble barrier.
"""
import sys

for _p in ("/opt/trn_rl_repo",):
    if _p not in sys.path:
        sys.path.append(_p)

import numpy as np

B, T, C = 4, 2048, 768
H, D = 12, 64
HL = H // 2          # 6 local heads
FL = HL * D          # 384 local features
NCT = C // 128       # 6 contraction tiles
NTT = T // 128       # 16 token tiles
QCH = 512            # q chunk (free dim of attention matmuls)
NQC = T // QCH       # 4 q chunks
VW = D + 1           # 65: V columns per head incl. ones column
EXP_SCALE = 2.0 / np.sqrt(D)  # reference uses logits = 2 * scores / sqrt(D)

_cache = {}
MM_TAGS = {}  # instruction name -> semantic tag (test-harness introspection)


def _build():
    import concourse.bass as bass
    import concourse.tile as tile
    from concourse import bacc, mybir

    f32 = mybir.dt.float32
    f32r = mybir.dt.float32r
    bf16 = mybir.dt.bfloat16
    Exp = mybir.ActivationFunctionType.Exp

    nc = bacc.Bacc("TRN2", target_bir_lowering=False, debug=False, num_devices=8)

    def mm(tag, *a, **kw):
        inst = nc.tensor.matmul(*a, **kw)
        MM_TAGS[inst.ins.name] = tag
        return inst

    xT = nc.dram_tensor("xT", [C, T], bf16, kind="ExternalInput").ap()
    wkqT = nc.dram_tensor("wkqT", [C, 2 * FL], bf16, kind="ExternalInput").ap()
    wvT = nc.dram_tensor("wvT", [C, FL], bf16, kind="ExternalInput").ap()
    biasT = nc.dram_tensor("bias", [128, 6 + FL], bf16, kind="ExternalInput").ap()
    wpT = nc.dram_tensor("wpT", [FL, C], bf16, kind="ExternalInput").ap()
    cmaskT = nc.dram_tensor("cmask", [128, 896], bf16, kind="ExternalInput").ap()
    out = nc.dram_tensor("out", [T, C], f32, kind="ExternalOutput").ap()

    with tile.TileContext(nc) as tc:
        from contextlib import ExitStack

        with ExitStack() as ctx:
            persist = ctx.enter_context(tc.tile_pool(name="persist", bufs=1))
            xpool = ctx.enter_context(tc.tile_pool(name="xchunk", bufs=2))
            ppool = ctx.enter_context(tc.tile_pool(name="ptile", bufs=6))
            lpool = ctx.enter_context(tc.tile_pool(name="linv", bufs=3))
            lrpool = ctx.enter_context(tc.tile_pool(name="linvrep", bufs=3))
            opool = ctx.enter_context(tc.tile_pool(name="outstg", bufs=3))
            # PSUM: psmm 2x1 banks + pss 2x2 + psy 2x1 = 8 banks
            ps_mm = ctx.enter_context(tc.tile_pool(name="psmm", bufs=2, space="PSUM"))
            ps_s = ctx.enter_context(tc.tile_pool(name="pss", bufs=2, space="PSUM"))
            ps_y = ctx.enter_context(tc.tile_pool(name="psy", bufs=2, space="PSUM"))

            # ---- persistent SBUF tensors ----
            kq_sb = persist.tile([128, 6, T], bf16)         # K^T (0-2) / Q^T (3-5)
            v_sb = persist.tile([128, NTT, HL * VW], bf16)  # token-major V + ones
            yn_sb = persist.tile([128, 3, T], bf16)         # normalized Y^T
            # wkq per c-tile, split into the f-tile-0/3 pair (first attention
            # unit's K/Q) and the rest, for early fine-grained DMA deps.
            wkqA_t = [
                persist.tile([128, 2, 128], bf16, name=f"wkqA{ci}")
                for ci in range(NCT)
            ]
            wkqB_t = [
                persist.tile([128, 4, 128], bf16, name=f"wkqB{ci}")
                for ci in range(NCT)
            ]
            wv_t = [
                persist.tile([128, FL], bf16, name=f"wv{ci}") for ci in range(NCT)
            ]
            wp_sb = persist.tile([128, 3, C], bf16)
            bias_sb = persist.tile([128, 6 + FL], bf16)     # bkq cols + bv replicated
            cmask_sb = persist.tile([128, 896], bf16)       # mask mi = [:, (3-mi)*128:+512]
            ones32 = persist.tile([1, 64], f32)
            nc.vector.memset(ones32, 1.0)
            ones_sb = persist.tile([1, 64], f32r)
            nc.vector.tensor_copy(ones_sb, ones32)

            def kq_w(fj):
                """stationary wkq slice for f-tile fj, c-tile ci."""
                if fj in (0, 3):
                    return lambda ci: wkqA_t[ci][:, 0 if fj == 0 else 1, :]
                idx = {1: 0, 2: 1, 4: 2, 5: 3}[fj]
                return lambda ci: wkqB_t[ci][:, idx, :]

            # ---- input DMAs, latency-ordered ----
            # Three FIFO DMA queues at ~100GB/s each. Priority: the f-tile-0/3
            # weights + x chunk 0 (first kq chains), then bias, wv + cmask
            # (first unit's V / masks), then the remaining kq weights, wp last.
            xT_r = xT.rearrange("(a p) t -> p a t", p=128)
            wkq_r = wkqT.rearrange("(a p) (g f) -> p a g f", p=128, f=128)
            wv_r = wvT.rearrange("(a p) f -> p a f", p=128)
            wp_r = wpT.rearrange("(a p) f -> p a f", p=128)
            x_tiles = {}

            def load_x(tn, engines=None):
                xt = [
                    xpool.tile([128, QCH], bf16, tag=f"x{ci}", name=f"xt{tn}_{ci}")
                    for ci in range(NCT)
                ]
                for ci in range(NCT):
                    eng = engines[ci] if engines else nc.sync
                    eng.dma_start(
                        out=xt[ci], in_=xT_r[:, ci, tn * QCH : (tn + 1) * QCH]
                    )
                x_tiles[tn] = xt

            def dma_wkqA(eng, ci):
                # f-tiles 0 and 3 of c-tile ci (columns 0:128 and 384:512)
                eng.dma_start(out=wkqA_t[ci][:, 0, :], in_=wkq_r[:, ci, 0, :])
                eng.dma_start(out=wkqA_t[ci][:, 1, :], in_=wkq_r[:, ci, 3, :])

            def dma_wkqB(eng, ci):
                for k, fj in enumerate((1, 2, 4, 5)):
                    eng.dma_start(out=wkqB_t[ci][:, k, :], in_=wkq_r[:, ci, fj, :])

            sy, sc, gp = nc.sync, nc.scalar, nc.gpsimd
            # interleave wkqA / x0 across queues so c-tiles land in consumption
            # order; each queue's list is its FIFO transfer order.
            dma_wkqA(sy, 0)
            load_x(0, engines=[sc, sy, gp, sc, sy, gp])
            dma_wkqA(sc, 1)
            dma_wkqA(gp, 2)
            dma_wkqA(sy, 3)
            dma_wkqA(sc, 4)
            dma_wkqA(gp, 5)
            sy.dma_start(out=bias_sb, in_=biasT)
            wv_engs = [sy, sc, gp, sy, sc, gp]
            for ci in range(NCT):
                wv_engs[ci].dma_start(out=wv_t[ci], in_=wv_r[:, ci, :])
            sc.dma_start(out=cmask_sb, in_=cmaskT)
            for ci, eng in enumerate([sy, sc, gp, sy, sc, gp]):
                dma_wkqB(eng, ci)
            # wp on gpsimd: free after startup, so wp never queues ahead of
            # the per-round x-chunk loads (sync) in a FIFO DMA queue.
            for fi in range(3):
                gp.dma_start(out=wp_sb[:, fi, :], in_=wp_r[:, fi, :])

            # ones columns of v_sb
            v4 = v_sb.rearrange("p t (h w) -> p t h w", h=HL)
            nc.vector.memset(v4[:, :, :, D : D + 1], 1.0)
            bv_rep = bias_sb[:, 6:].rearrange("p (h d) -> p h d", h=HL)
            # tensor_scalar_add requires an f32 scalar operand
            bkq_f32 = persist.tile([128, 6], f32)
            nc.vector.tensor_copy(bkq_f32, bias_sb[:, 0:6])

            def kq_chain(tn, fj):
                xt = x_tiles[tn]
                w = kq_w(fj)
                ps = ps_mm.tile([128, QCH], f32, tag="psmm", name=f"kq{tn}_{fj}")
                for ci in range(NCT):
                    mm(
                        f"kq{tn}.{fj}.{ci}",
                        ps,
                        lhsT=w(ci),
                        rhs=xt[ci],
                        start=(ci == 0),
                        stop=(ci == NCT - 1),
                    )
                    yield
                nc.vector.tensor_scalar_add(
                    kq_sb[:, fj, tn * QCH : (tn + 1) * QCH],
                    ps,
                    bkq_f32[:, fj : fj + 1],
                )

            def v_chain(tn, k4):
                xt = x_tiles[tn]
                kt = tn * 4 + k4
                ps = ps_mm.tile([128, FL], f32, tag="psmm", name=f"v{kt}")
                for ci in range(NCT):
                    mm(
                        f"v{kt}.{ci}",
                        ps,
                        lhsT=xt[ci][:, k4 * 128 : (k4 + 1) * 128],
                        rhs=wv_t[ci],
                        start=(ci == 0),
                        stop=(ci == NCT - 1),
                    )
                    yield
                nc.vector.tensor_add(
                    v4[:, kt, :, 0:D],
                    ps.rearrange("p (h d) -> p h d", h=HL),
                    bv_rep,
                )

            def qkv_fill(tn):
                """deadline-tagged chains for chunk tn (fj0/fj3 handled
                separately for tn=0). Deadline (tn, h): must be fully
                emitted before unit (tn, h) is opened."""
                chains = []
                if tn > 0:
                    chains += [(tn, 0, kq_chain(tn, 0)), (tn, 0, kq_chain(tn, 3))]
                chains += [(tn, 0, v_chain(tn, k4)) for k4 in range(4)]
                chains += [
                    (tn, 2, kq_chain(tn, 1)),
                    (tn, 2, kq_chain(tn, 4)),
                    (tn, 4, kq_chain(tn, 2)),
                    (tn, 4, kq_chain(tn, 5)),
                ]
                return chains

            def proj_tile(qt):
                ostg = opool.tile([128, C], f32, tag="outstg", name=f"o{qt}")
                for cj in range(2):
                    ps = ps_mm.tile(
                        [128, FL], f32, tag="psmm", name=f"pj{qt}_{cj}"
                    )
                    for fi in range(3):
                        mm(
                            f"pj{qt}.{cj}.{fi}",
                            ps,
                            lhsT=yn_sb[:, fi, qt * 128 : (qt + 1) * 128],
                            rhs=wp_sb[:, fi, cj * FL : (cj + 1) * FL],
                            start=(fi == 0),
                            stop=(fi == 2),
                        )
                        yield
                    nc.vector.tensor_copy(ostg[:, cj * FL : (cj + 1) * FL], ps)
                nc.sync.dma_start(out=out[qt * 128 : (qt + 1) * 128, :], in_=ostg)

            # ---- filler stepper: singleton active generator ----------------
            # At most one filler generator is mid-flight at any emission point
            # (+1 broadcast psum tile) so the 2-buf psmm pool can never
            # deadlock the in-order PE stream.
            q_fill = []   # [(tn, dh, gen)] deadline-ordered qkv chains
            p_fill = []   # [(tc, gen)] projection tiles, tc = source chunk
            state = {"cur": None, "qc": 0, "h": 0}

            def exhaust(gen):
                for _ in gen:
                    pass

            def next_gen():
                qc, h = state["qc"], state["h"]
                # a chunk-tc proj tile is eligible once a full unit has passed
                # since finish(tc,5) was emitted (at unit (tc+1,0)): its fi=2
                # matmul must not chase a Vector chain still in flight.
                p_ok = p_fill and (qc, h) >= (p_fill[0][0] + 1, 2)
                if p_ok and (qc == NQC - 1 or h < 2 or not q_fill):
                    return (None, p_fill.pop(0)[1])
                if q_fill:
                    tn, dh, gen = q_fill.pop(0)
                    return ((tn, dh), gen)
                return None

            def fill_step(n):
                for _ in range(n):
                    if state["cur"] is None:
                        state["cur"] = next_gen()
                        if state["cur"] is None:
                            return
                    try:
                        next(state["cur"][1])
                    except StopIteration:
                        state["cur"] = None

            def drain_due(qc, h):
                cur = state["cur"]
                if cur is not None and cur[0] is not None and cur[0] <= (qc, h):
                    exhaust(cur[1])
                    state["cur"] = None
                while q_fill and (q_fill[0][0], q_fill[0][1]) <= (qc, h):
                    exhaust(q_fill.pop(0)[2])

            def attn_unit(qc, h):
                hp, hi = h % 2, h // 2
                kmax = (qc + 1) * 4
                psy = ps_y.tile([128, QCH], f32, tag="psy", name=f"y{qc}_{h}")
                # software-pipelined: emit S-pair(kp) before Y-pair(kp-1) so the
                # in-order PE stream never waits on the exp of the current pair
                npairs = kmax // 2
                pts = {}

                def s_pair(kp):
                    pss = ps_s.tile(
                        [128, 2 * QCH], f32, tag="pss", name=f"s{qc}_{h}_{kp}"
                    )
                    for ki in range(2):
                        kt = 2 * kp + ki
                        mm(
                            f"s{qc}.{h}.{kt}",
                            pss[:, ki * QCH : (ki + 1) * QCH],
                            lhsT=kq_sb[
                                hp * 64 : hp * 64 + 64,
                                hi,
                                kt * 128 : (kt + 1) * 128,
                            ],
                            rhs=kq_sb[
                                hp * 64 : hp * 64 + 64,
                                3 + hi,
                                qc * QCH : (qc + 1) * QCH,
                            ],
                            start=True,
                            stop=True,
                        )
                    pt = ppool.tile(
                        [128, 2 * QCH], bf16, tag="ptile", name=f"p{qc}_{h}_{kp}"
                    )
                    nc.scalar.activation(pt, pss, Exp, scale=float(EXP_SCALE))
                    pts[kp] = pt

                def y_pair(kp):
                    pt = pts.pop(kp)
                    for ki in range(2):
                        kt = 2 * kp + ki
                        mi = kt - qc * 4
                        if mi >= 0:  # diagonal tile -> apply causal mask
                            nc.vector.tensor_mul(
                                pt[:, ki * QCH : (ki + 1) * QCH],
                                pt[:, ki * QCH : (ki + 1) * QCH],
                                cmask_sb[:, (3 - mi) * 128 : (3 - mi) * 128 + QCH],
                            )
                        mm(
                            f"y{qc}.{h}.{kt}",
                            psy[0:VW, :],
                            lhsT=v_sb[:, kt, h * VW : (h + 1) * VW],
                            rhs=pt[:, ki * QCH : (ki + 1) * QCH],
                            start=(kt == 0),
                            stop=(kt == kmax - 1),
                        )

                def rest():
                    for kp in range(2, npairs):
                        s_pair(kp)
                        fill_step(1)
                        y_pair(kp - 2)
                        fill_step(1)
                    y_pair(npairs - 2)
                    fill_step(1)
                    y_pair(npairs - 1)

                # prologue: two S-pairs issued now so ACT has exp work while
                # the previous unit's finish + filler chains occupy PE
                s_pair(0)
                s_pair(1)
                return psy, rest

            def attn_finish(qc, h, psy):
                hp, hi = h % 2, h // 2
                # softmax denominator: lrow -> 1/l -> f32r -> broadcast to 64 rows
                lrow = lpool.tile([1, QCH], f32, tag="lrow", name=f"lr{qc}_{h}")
                nc.vector.tensor_copy(lrow, psy[D : D + 1, :])
                linv32 = lpool.tile([1, QCH], f32, tag="linv32", name=f"li{qc}_{h}")
                nc.vector.reciprocal_approx_fast(out=linv32, in_=lrow)
                linv = lpool.tile([1, QCH], f32r, tag="linv", name=f"lv{qc}_{h}")
                nc.vector.tensor_copy(linv, linv32)
                psb = ps_mm.tile([128, QCH], f32, tag="psmm", name=f"lb{qc}_{h}")
                mm(
                    f"bc{qc}.{h}",
                    psb[0:64, :],
                    lhsT=ones_sb,
                    rhs=linv,
                    start=True,
                    stop=True,
                )
                lrep = lrpool.tile([64, QCH], f32, tag="linvrep", name=f"lp{qc}_{h}")
                nc.vector.tensor_copy(lrep, psb[0:64, :])
                nc.vector.tensor_mul(
                    yn_sb[hp * 64 : hp * 64 + 64, hi, qc * QCH : (qc + 1) * QCH],
                    psy[0:D, :],
                    lrep,
                )

            # ---- pipelined emission ----
            # chunk-0 K/Q f-tile 0/3 chains emitted inline (unit (0,0)'s S
            # reads them); everything else goes through the filler stepper.
            exhaust(kq_chain(0, 0))
            exhaust(kq_chain(0, 3))
            q_fill.extend(qkv_fill(0))

            pending = None
            for qc in range(NQC):
                state["qc"] = qc
                if qc + 1 < NQC:
                    load_x(qc + 1)
                    q_fill.extend(qkv_fill(qc + 1))
                for h in range(HL):
                    state["h"] = h
                    drain_due(qc, h)
                    psy, rest = attn_unit(qc, h)
                    if pending is not None:
                        attn_finish(pending[0], pending[1], pending[2])
                        fill_step(4)
                    rest()
                    # unit-end quota: early rounds have few attention pairs to
                    # hide filler in, so their next-chunk qkv needs a top-up
                    # (emitted while the next unit's prologue feeds ACT).
                    fill_step((7, 3, 2, 2)[qc])
                    pending = (qc, h, psy)
                p_fill.extend((qc, proj_tile(qc * 4 + q4)) for q4 in range(4))
            attn_finish(pending[0], pending[1], pending[2])
            if state["cur"] is not None:
                exhaust(state["cur"][1])
                state["cur"] = None
            for _, _, g in q_fill:
                exhaust(g)
            q_fill.clear()
            for _, g in p_fill:
                exhaust(g)
            p_fill.clear()

    nc.compile()
    return nc


def _shard_inputs(x, w_attn, b_attn, w_proj, b_proj):
    import ml_dtypes

    bf16 = ml_dtypes.bfloat16
    # causal-mask base [128, 896]: mask for diagonal offset mi is the slice
    # [:, (3-mi)*128 : (3-mi)*128+512]; element (p, f) keeps iff f-p >= mi*128.
    p = np.arange(128)[:, None]
    g = np.arange(896)[None, :]
    cmask = ((g - p) >= 384).astype(bf16)
    in_maps = []
    for core in range(8):
        b, hg = core // 2, core % 2
        hs = hg * FL
        k_w = w_attn[hs : hs + FL]
        q_w = w_attn[C + hs : C + hs + FL]
        v_w = w_attn[2 * C + hs : 2 * C + hs + FL]
        bkq = np.concatenate(
            [b_attn[hs : hs + FL], b_attn[C + hs : C + hs + FL]]
        ).astype(np.float32)
        bv = b_attn[2 * C + hs : 2 * C + hs + FL].astype(np.float32)
        bias = np.concatenate(
            [bkq.reshape(6, 128).T, np.broadcast_to(bv, (128, FL))], axis=1
        )
        in_maps.append(
            {
                "xT": np.ascontiguousarray(x[b].T).astype(bf16),
                "wkqT": np.ascontiguousarray(
                    np.concatenate([k_w, q_w], axis=0).T
                ).astype(bf16),
                "wvT": np.ascontiguousarray(v_w.T).astype(bf16),
                "bias": np.ascontiguousarray(bias).astype(bf16),
                "wpT": np.ascontiguousarray(w_proj[:, hs : hs + FL].T).astype(bf16),
                "cmask": cmask,
            }
        )
    return in_maps


def _run(inputs, trace=False, trace_kwargs=None):
    from concourse.bass_utils import run_bass_kernel_spmd

    if "nc" not in _cache:
        _cache["nc"] = _build()
    nc = _cache["nc"]
    in_maps = _shard_inputs(**inputs)
    kw = {}
    if trace:
        kw["trace"] = True
        if trace_kwargs:
            kw.update(trace_kwargs)
    res = run_bass_kernel_spmd(nc, in_maps, core_ids=list(range(8)), **kw)
    x = inputs["x"]
    outf = np.empty((B, T, C), dtype=np.float32)
    for b in range(B):
        outf[b] = (
            res.results[2 * b]["out"]
            + res.results[2 * b + 1]["out"]
            + inputs["b_proj"]
        )
    return outf, res


def kernel(x, w_attn, b_attn, w_proj, b_proj):
    x = np.asarray(x, dtype=np.float32)
    w_attn = np.asarray(w_attn, dtype=np.float32)
    b_attn = np.asarray(b_attn, dtype=np.float32)
    w_proj = np.asarray(w_proj, dtype=np.float32)
    b_proj = np.asarray(b_proj, dtype=np.float32)
    assert x.shape == (B, T, C), x.shape
    outf, _ = _run(
        dict(x=x, w_attn=w_attn, b_attn=b_attn, w_proj=w_proj, b_proj=b_proj)
    )
    return outf


# revision 17
# speedup vs baseline: 1.0630x; 1.0630x over previous
"""Causal self-attention (B=4, T=2048, C=768, H=12, D=64) on 8 TRN2 NeuronCores.

Sharding: core = 2*b + hg. Data parallel over batch (4), tensor parallel over
heads (2 groups of 6). Each core computes qkv for its 6 heads, causal
attention, and a partial output projection (its heads' columns of w_proj);
the host sums the two partials per batch and adds b_proj.

Layout notes (per core):
  - xT   [768, 2048]  x[b] transposed on host (contraction dim on partitions)
  - kq   [128, 6, 2048] SBUF: f-tiles 0-2 = K^T feats, 3-5 = Q^T feats
  - v    [128, 16, 390] SBUF: token-major V, 65 cols/head (col 64 = ones so
         the attn@V matmul also produces the softmax denominator l)
  - scores computed transposed S^T[k, q] so no transposes are needed anywhere;
    softmax uses no max-subtraction (logits are O(10) for this problem) so
    P = exp(0.25 * QK^T_raw), Y^T_aug = V_aug^T @ P^T accumulated over k-tiles.
  - matmul operands are bf16 (fp32 PSUM accumulation); 1/l via a custom-DVE
    approx reciprocal (SBUF-in only!) + tiny f32r ones-outer-product broadcast.
  - S^T tiles are paired [128, 1024] so one ACT exp covers two k-tiles.

Scheduling (the performance core of this kernel):
  - The attention inner loop is ACT-paced: one exp pair is ~1.09us on the
    Activation engine vs 0.86us for its 4 matmuls (S-pair + Y-pair), so the
    PE needs ~1 independent filler matmul per half-pair to stay dense.
    Fillers (qkv for chunk qc+1, projection for finished chunks) are emitted
    one matmul at a time INSIDE each attention unit via a singleton generator
    stepper (never two fillers mid-flight -> bounded PSUM pool usage).
  - qkv chains carry deadlines (chunk tn's K/Q f-tile pair (fj,3+fj) is first
    read by unit (tn, 2*fj); V by unit (tn,0)) and are force-drained right
    before the unit that needs them, so a late filler can never end up behind
    the matmul that consumes it in the in-order PE stream.
  - startup: biases (bf16, replicated host-side) + causal mask come as plain
    DMA inputs; wkq is split per c-tile into an f-tile-0/3 part (needed by the
    first attention unit) and the rest, and the three DMA queues (~100GB/s
    each, FIFO) are ordered so the first kq chain is fed within ~2us of the
    preamble barrier.
"""
import sys

for _p in ("/opt/trn_rl_repo",):
    if _p not in sys.path:
        sys.path.append(_p)

import numpy as np

B, T, C = 4, 2048, 768
H, D = 12, 64
HL = H // 2          # 6 local heads
FL = HL * D          # 384 local features
NCT = C // 128       # 6 contraction tiles
NTT = T // 128       # 16 token tiles
QCH = 512            # q chunk (free dim of attention matmuls)
NQC = T // QCH       # 4 q chunks
VW = D + 1           # 65: V columns per head incl. ones column
EXP_SCALE = 2.0 / np.sqrt(D)  # reference uses logits = 2 * scores / sqrt(D)

_cache = {}
MM_TAGS = {}  # instruction name -> semantic tag (test-harness introspection)


def _build():
    import concourse.bass as bass
    import concourse.tile as tile
    from concourse import bacc, mybir

    f32 = mybir.dt.float32
    f32r = mybir.dt.float32r
    bf16 = mybir.dt.bfloat16
    Exp = mybir.ActivationFunctionType.Exp

    nc = bacc.Bacc("TRN2", target_bir_lowering=False, debug=False, num_devices=8)

    def mm(tag, *a, **kw):
        inst = nc.tensor.matmul(*a, **kw)
        MM_TAGS[inst.ins.name] = tag
        return inst

    xT = nc.dram_tensor("xT", [C, T], bf16, kind="ExternalInput").ap()
    wkqT = nc.dram_tensor("wkqT", [C, 2 * FL], bf16, kind="ExternalInput").ap()
    wvT = nc.dram_tensor("wvT", [C, FL], bf16, kind="ExternalInput").ap()
    biasT = nc.dram_tensor("bias", [128, 6 + FL], bf16, kind="ExternalInput").ap()
    wpT = nc.dram_tensor("wpT", [FL, C], bf16, kind="ExternalInput").ap()
    cmaskT = nc.dram_tensor("cmask", [128, 896], bf16, kind="ExternalInput").ap()
    out = nc.dram_tensor("out", [T, C], f32, kind="ExternalOutput").ap()

    with tile.TileContext(nc) as tc:
        from contextlib import ExitStack

        with ExitStack() as ctx:
            persist = ctx.enter_context(tc.tile_pool(name="persist", bufs=1))
            xpool = ctx.enter_context(tc.tile_pool(name="xchunk", bufs=2))
            ppool = ctx.enter_context(tc.tile_pool(name="ptile", bufs=6))
            lpool = ctx.enter_context(tc.tile_pool(name="linv", bufs=3))
            lrpool = ctx.enter_context(tc.tile_pool(name="linvrep", bufs=3))
            opool = ctx.enter_context(tc.tile_pool(name="outstg", bufs=3))
            # PSUM: psmm 2x1 banks + pss 2x2 + psy 2x1 = 8 banks
            ps_mm = ctx.enter_context(tc.tile_pool(name="psmm", bufs=2, space="PSUM"))
            ps_s = ctx.enter_context(tc.tile_pool(name="pss", bufs=2, space="PSUM"))
            ps_y = ctx.enter_context(tc.tile_pool(name="psy", bufs=2, space="PSUM"))

            # ---- persistent SBUF tensors ----
            kq_sb = persist.tile([128, 6, T], bf16)         # K^T (0-2) / Q^T (3-5)
            v_sb = persist.tile([128, NTT, HL * VW], bf16)  # token-major V + ones
            yn_sb = persist.tile([128, 3, T], bf16)         # normalized Y^T
            # wkq per c-tile, split into the f-tile-0/3 pair (first attention
            # unit's K/Q) and the rest, for early fine-grained DMA deps.
            wkqA_t = [
                persist.tile([128, 2, 128], bf16, name=f"wkqA{ci}")
                for ci in range(NCT)
            ]
            wkqB_t = [
                persist.tile([128, 4, 128], bf16, name=f"wkqB{ci}")
                for ci in range(NCT)
            ]
            wv_t = [
                persist.tile([128, FL], bf16, name=f"wv{ci}") for ci in range(NCT)
            ]
            wp_sb = persist.tile([128, 3, C], bf16)
            bias_sb = persist.tile([128, 6 + FL], bf16)     # bkq cols + bv replicated
            cmask_sb = persist.tile([128, 896], bf16)       # mask mi = [:, (3-mi)*128:+512]
            ones32 = persist.tile([1, 64], f32)
            nc.vector.memset(ones32, 1.0)
            ones_sb = persist.tile([1, 64], f32r)
            nc.vector.tensor_copy(ones_sb, ones32)

            def kq_w(fj):
                """stationary wkq slice for f-tile fj, c-tile ci."""
                if fj in (0, 3):
                    return lambda ci: wkqA_t[ci][:, 0 if fj == 0 else 1, :]
                idx = {1: 0, 2: 1, 4: 2, 5: 3}[fj]
                return lambda ci: wkqB_t[ci][:, idx, :]

            # ---- input DMAs, latency-ordered ----
            # Three FIFO DMA queues at ~100GB/s each. Priority: the f-tile-0/3
            # weights + x chunk 0 (first kq chains), then bias, wv + cmask
            # (first unit's V / masks), then the remaining kq weights, wp last.
            xT_r = xT.rearrange("(a p) t -> p a t", p=128)
            wkq_r = wkqT.rearrange("(a p) (g f) -> p a g f", p=128, f=128)
            wv_r = wvT.rearrange("(a p) f -> p a f", p=128)
            wp_r = wpT.rearrange("(a p) f -> p a f", p=128)
            x_tiles = {}

            def load_x(tn, engines=None):
                xt = [
                    xpool.tile([128, QCH], bf16, tag=f"x{ci}", name=f"xt{tn}_{ci}")
                    for ci in range(NCT)
                ]
                for ci in range(NCT):
                    eng = engines[ci] if engines else nc.sync
                    eng.dma_start(
                        out=xt[ci], in_=xT_r[:, ci, tn * QCH : (tn + 1) * QCH]
                    )
                x_tiles[tn] = xt

            def dma_wkqA(eng, ci):
                # f-tiles 0 and 3 of c-tile ci (columns 0:128 and 384:512)
                eng.dma_start(out=wkqA_t[ci][:, 0, :], in_=wkq_r[:, ci, 0, :])
                eng.dma_start(out=wkqA_t[ci][:, 1, :], in_=wkq_r[:, ci, 3, :])

            def dma_wkqB(eng, ci):
                for k, fj in enumerate((1, 2, 4, 5)):
                    eng.dma_start(out=wkqB_t[ci][:, k, :], in_=wkq_r[:, ci, fj, :])

            sy, sc, gp = nc.sync, nc.scalar, nc.gpsimd
            # interleave wkqA / x0 across queues so c-tiles land in consumption
            # order; each queue's list is its FIFO transfer order.
            dma_wkqA(sy, 0)
            load_x(0, engines=[sc, sy, gp, sc, sy, gp])
            dma_wkqA(sc, 1)
            dma_wkqA(gp, 2)
            dma_wkqA(sy, 3)
            dma_wkqA(sc, 4)
            dma_wkqA(gp, 5)
            sy.dma_start(out=bias_sb, in_=biasT)
            wv_engs = [sy, sc, gp, sy, sc, gp]
            for ci in range(NCT):
                wv_engs[ci].dma_start(out=wv_t[ci], in_=wv_r[:, ci, :])
            sc.dma_start(out=cmask_sb, in_=cmaskT)
            for ci, eng in enumerate([sy, sc, gp, sy, sc, gp]):
                dma_wkqB(eng, ci)
            # wp on gpsimd: free after startup, so wp never queues ahead of
            # the per-round x-chunk loads (sync) in a FIFO DMA queue.
            for fi in range(3):
                gp.dma_start(out=wp_sb[:, fi, :], in_=wp_r[:, fi, :])

            # ones columns of v_sb
            v4 = v_sb.rearrange("p t (h w) -> p t h w", h=HL)
            nc.vector.memset(v4[:, :, :, D : D + 1], 1.0)
            bv_rep = bias_sb[:, 6:].rearrange("p (h d) -> p h d", h=HL)
            # tensor_scalar_add requires an f32 scalar operand
            bkq_f32 = persist.tile([128, 6], f32)
            nc.vector.tensor_copy(bkq_f32, bias_sb[:, 0:6])

            def kq_chain(tn, fj):
                xt = x_tiles[tn]
                w = kq_w(fj)
                ps = ps_mm.tile([128, QCH], f32, tag="psmm", name=f"kq{tn}_{fj}")
                for ci in range(NCT):
                    mm(
                        f"kq{tn}.{fj}.{ci}",
                        ps,
                        lhsT=w(ci),
                        rhs=xt[ci],
                        start=(ci == 0),
                        stop=(ci == NCT - 1),
                    )
                    yield
                nc.vector.tensor_scalar_add(
                    kq_sb[:, fj, tn * QCH : (tn + 1) * QCH],
                    ps,
                    bkq_f32[:, fj : fj + 1],
                )

            def v_chain(tn, k4):
                xt = x_tiles[tn]
                kt = tn * 4 + k4
                ps = ps_mm.tile([128, FL], f32, tag="psmm", name=f"v{kt}")
                for ci in range(NCT):
                    mm(
                        f"v{kt}.{ci}",
                        ps,
                        lhsT=xt[ci][:, k4 * 128 : (k4 + 1) * 128],
                        rhs=wv_t[ci],
                        start=(ci == 0),
                        stop=(ci == NCT - 1),
                    )
                    yield
                nc.vector.tensor_add(
                    v4[:, kt, :, 0:D],
                    ps.rearrange("p (h d) -> p h d", h=HL),
                    bv_rep,
                )

            def qkv_fill(tn):
                """deadline-tagged chains for chunk tn (fj0/fj3 handled
                separately for tn=0). Deadline (tn, h): must be fully
                emitted before unit (tn, h) is opened."""
                chains = []
                if tn > 0:
                    chains += [(tn, 0, kq_chain(tn, 0)), (tn, 0, kq_chain(tn, 3))]
                chains += [(tn, 0, v_chain(tn, k4)) for k4 in range(4)]
                chains += [
                    (tn, 2, kq_chain(tn, 1)),
                    (tn, 2, kq_chain(tn, 4)),
                    (tn, 4, kq_chain(tn, 2)),
                    (tn, 4, kq_chain(tn, 5)),
                ]
                return chains

            def proj_tile(qt):
                ostg = opool.tile([128, C], f32, tag="outstg", name=f"o{qt}")
                for cj in range(2):
                    ps = ps_mm.tile(
                        [128, FL], f32, tag="psmm", name=f"pj{qt}_{cj}"
                    )
                    for fi in range(3):
                        mm(
                            f"pj{qt}.{cj}.{fi}",
                            ps,
                            lhsT=yn_sb[:, fi, qt * 128 : (qt + 1) * 128],
                            rhs=wp_sb[:, fi, cj * FL : (cj + 1) * FL],
                            start=(fi == 0),
                            stop=(fi == 2),
                        )
                        yield
                    nc.vector.tensor_copy(ostg[:, cj * FL : (cj + 1) * FL], ps)
                nc.sync.dma_start(out=out[qt * 128 : (qt + 1) * 128, :], in_=ostg)

            # ---- filler stepper: singleton active generator ----------------
            # At most one filler generator is mid-flight at any emission point
            # (+1 broadcast psum tile) so the 2-buf psmm pool can never
            # deadlock the in-order PE stream.
            q_fill = []   # [(tn, dh, gen)] deadline-ordered qkv chains
            p_fill = []   # [(tc, gen)] projection tiles, tc = source chunk
            state = {"cur": None, "qc": 0, "h": 0}

            def exhaust(gen):
                for _ in gen:
                    pass

            def next_gen():
                qc, h = state["qc"], state["h"]
                # a chunk-tc proj tile is eligible once a full unit has passed
                # since finish(tc,5) was emitted (at unit (tc+1,0)): its fi=2
                # matmul must not chase a Vector chain still in flight.
                p_ok = p_fill and (qc, h) >= (p_fill[0][0] + 1, 2)
                if p_ok and (qc == NQC - 1 or not q_fill):
                    return (None, p_fill.pop(0)[1])
                if q_fill:
                    tn, dh, gen = q_fill.pop(0)
                    return ((tn, dh), gen)
                return None

            def fill_step(n):
                for _ in range(n):
                    if state["cur"] is None:
                        state["cur"] = next_gen()
                        if state["cur"] is None:
                            return
                    try:
                        next(state["cur"][1])
                    except StopIteration:
                        state["cur"] = None

            def drain_due(qc, h):
                cur = state["cur"]
                if cur is not None and cur[0] is not None and cur[0] <= (qc, h):
                    exhaust(cur[1])
                    state["cur"] = None
                while q_fill and (q_fill[0][0], q_fill[0][1]) <= (qc, h):
                    exhaust(q_fill.pop(0)[2])

            def attn_unit(qc, h):
                hp, hi = h % 2, h // 2
                kmax = (qc + 1) * 4
                psy = ps_y.tile([128, QCH], f32, tag="psy", name=f"y{qc}_{h}")
                # software-pipelined: emit S-pair(kp) before Y-pair(kp-1) so the
                # in-order PE stream never waits on the exp of the current pair
                npairs = kmax // 2
                pts = {}

                def s_pair(kp):
                    pss = ps_s.tile(
                        [128, 2 * QCH], f32, tag="pss", name=f"s{qc}_{h}_{kp}"
                    )
                    for ki in range(2):
                        kt = 2 * kp + ki
                        mm(
                            f"s{qc}.{h}.{kt}",
                            pss[:, ki * QCH : (ki + 1) * QCH],
                            lhsT=kq_sb[
                                hp * 64 : hp * 64 + 64,
                                hi,
                                kt * 128 : (kt + 1) * 128,
                            ],
                            rhs=kq_sb[
                                hp * 64 : hp * 64 + 64,
                                3 + hi,
                                qc * QCH : (qc + 1) * QCH,
                            ],
                            start=True,
                            stop=True,
                        )
                    pt = ppool.tile(
                        [128, 2 * QCH], bf16, tag="ptile", name=f"p{qc}_{h}_{kp}"
                    )
                    nc.scalar.activation(pt, pss, Exp, scale=float(EXP_SCALE))
                    pts[kp] = pt

                def y_pair(kp):
                    pt = pts.pop(kp)
                    for ki in range(2):
                        kt = 2 * kp + ki
                        mi = kt - qc * 4
                        if mi >= 0:  # diagonal tile -> apply causal mask
                            nc.vector.tensor_mul(
                                pt[:, ki * QCH : (ki + 1) * QCH],
                                pt[:, ki * QCH : (ki + 1) * QCH],
                                cmask_sb[:, (3 - mi) * 128 : (3 - mi) * 128 + QCH],
                            )
                        mm(
                            f"y{qc}.{h}.{kt}",
                            psy[0:VW, :],
                            lhsT=v_sb[:, kt, h * VW : (h + 1) * VW],
                            rhs=pt[:, ki * QCH : (ki + 1) * QCH],
                            start=(kt == 0),
                            stop=(kt == kmax - 1),
                        )

                def rest():
                    for kp in range(2, npairs):
                        s_pair(kp)
                        y_pair(kp - 2)
                    y_pair(npairs - 2)
                    y_pair(npairs - 1)

                # prologue: two S-pairs issued now so ACT has exp work while
                # the previous unit's finish + filler chains occupy PE
                s_pair(0)
                s_pair(1)
                return psy, rest

            def attn_finish(qc, h, psy):
                hp, hi = h % 2, h // 2
                # softmax denominator: lrow -> 1/l -> f32r -> broadcast to 64 rows
                lrow = lpool.tile([1, QCH], f32, tag="lrow", name=f"lr{qc}_{h}")
                nc.vector.tensor_copy(lrow, psy[D : D + 1, :])
                linv32 = lpool.tile([1, QCH], f32, tag="linv32", name=f"li{qc}_{h}")
                nc.vector.reciprocal_approx_fast(out=linv32, in_=lrow)
                linv = lpool.tile([1, QCH], f32r, tag="linv", name=f"lv{qc}_{h}")
                nc.vector.tensor_copy(linv, linv32)
                psb = ps_mm.tile([128, QCH], f32, tag="psmm", name=f"lb{qc}_{h}")
                mm(
                    f"bc{qc}.{h}",
                    psb[0:64, :],
                    lhsT=ones_sb,
                    rhs=linv,
                    start=True,
                    stop=True,
                )
                lrep = lrpool.tile([64, QCH], f32, tag="linvrep", name=f"lp{qc}_{h}")
                nc.vector.tensor_copy(lrep, psb[0:64, :])
                nc.vector.tensor_mul(
                    yn_sb[hp * 64 : hp * 64 + 64, hi, qc * QCH : (qc + 1) * QCH],
                    psy[0:D, :],
                    lrep,
                )

            # ---- pipelined emission ----
            # chunk-0 K/Q f-tile 0/3 chains emitted inline (unit (0,0)'s S
            # reads them); everything else goes through the filler stepper.
            exhaust(kq_chain(0, 0))
            exhaust(kq_chain(0, 3))
            q_fill.extend(qkv_fill(0))

            pending = None
            for qc in range(NQC):
                state["qc"] = qc
                if qc + 1 < NQC:
                    load_x(qc + 1)
                    q_fill.extend(qkv_fill(qc + 1))
                for h in range(HL):
                    state["h"] = h
                    drain_due(qc, h)
                    psy, rest = attn_unit(qc, h)
                    if pending is not None:
                        attn_finish(pending[0], pending[1], pending[2])
                    # one filler clump per unit, right after the prologue: it
                    # executes while ACT catches up on the previous unit's
                    # tail exps + this prologue (the PE would otherwise stall
                    # on the pss-bank recycle of the first rest() S-pair).
                    # A clump keeps lhsT shapes uniform (single-matmul
                    # interleave pays ~100ns of LDWEIGHTS context switch per
                    # filler, measured to cancel the filler's own value).
                    fill_step((13, 13, 13, 16)[qc])
                    rest()
                    pending = (qc, h, psy)
                p_fill.extend((qc, proj_tile(qc * 4 + q4)) for q4 in range(4))
            attn_finish(pending[0], pending[1], pending[2])
            if state["cur"] is not None:
                exhaust(state["cur"][1])
                state["cur"] = None
            for _, _, g in q_fill:
                exhaust(g)
            q_fill.clear()
            for _, g in p_fill:
                exhaust(g)
            p_fill.clear()

    nc.compile()
    return nc


def _shard_inputs(x, w_attn, b_attn, w_proj, b_proj):
    import ml_dtypes

    bf16 = ml_dtypes.bfloat16
    # causal-mask base [128, 896]: mask for diagonal offset mi is the slice
    # [:, (3-mi)*128 : (3-mi)*128+512]; element (p, f) keeps iff f-p >= mi*128.
    p = np.arange(128)[:, None]
    g = np.arange(896)[None, :]
    cmask = ((g - p) >= 384).astype(bf16)
    in_maps = []
    for core in range(8):
        b, hg = core // 2, core % 2
        hs = hg * FL
        k_w = w_attn[hs : hs + FL]
        q_w = w_attn[C + hs : C + hs + FL]
        v_w = w_attn[2 * C + hs : 2 * C + hs + FL]
        bkq = np.concatenate(
            [b_attn[hs : hs + FL], b_attn[C + hs : C + hs + FL]]
        ).astype(np.float32)
        bv = b_attn[2 * C + hs : 2 * C + hs + FL].astype(np.float32)
        bias = np.concatenate(
            [bkq.reshape(6, 128).T, np.broadcast_to(bv, (128, FL))], axis=1
        )
        in_maps.append(
            {
                "xT": np.ascontiguousarray(x[b].T).astype(bf16),
                "wkqT": np.ascontiguousarray(
                    np.concatenate([k_w, q_w], axis=0).T
                ).astype(bf16),
                "wvT": np.ascontiguousarray(v_w.T).astype(bf16),
                "bias": np.ascontiguousarray(bias).astype(bf16),
                "wpT": np.ascontiguousarray(w_proj[:, hs : hs + FL].T).astype(bf16),
                "cmask": cmask,
            }
        )
    return in_maps


def _run(inputs, trace=False, trace_kwargs=None):
    from concourse.bass_utils import run_bass_kernel_spmd

    if "nc" not in _cache:
        _cache["nc"] = _build()
    nc = _cache["nc"]
    in_maps = _shard_inputs(**inputs)
    kw = {}
    if trace:
        kw["trace"] = True
        if trace_kwargs:
            kw.update(trace_kwargs)
    res = run_bass_kernel_spmd(nc, in_maps, core_ids=list(range(8)), **kw)
    x = inputs["x"]
    outf = np.empty((B, T, C), dtype=np.float32)
    for b in range(B):
        outf[b] = (
            res.results[2 * b]["out"]
            + res.results[2 * b + 1]["out"]
            + inputs["b_proj"]
        )
    return outf, res


def kernel(x, w_attn, b_attn, w_proj, b_proj):
    x = np.asarray(x, dtype=np.float32)
    w_attn = np.asarray(w_attn, dtype=np.float32)
    b_attn = np.asarray(b_attn, dtype=np.float32)
    w_proj = np.asarray(w_proj, dtype=np.float32)
    b_proj = np.asarray(b_proj, dtype=np.float32)
    assert x.shape == (B, T, C), x.shape
    outf, _ = _run(
        dict(x=x, w_attn=w_attn, b_attn=b_attn, w_proj=w_proj, b_proj=b_proj)
    )
    return outf
